# revision 1
# baseline (speedup 1.0000x reference)
"""Trainium2 Bass kernel for nn_ConceptEmbedding (type-conditioned embedding lookup).

Reference computation (per token position (b, s)):
    t = token_type[b, s]
    out[b, s, :] = proc_emb[concept]  if t == 1
                   med_emb[concept]   if t == 2
                   chart_emb[concept] if t == 3
                   0                  otherwise

Strategy (v3):
  - Fold the three tables into one [3V, E] table with flat row index
    (t-1)*V + concept. Tokens with t outside {1,2,3} produce zeros and are
    never sent to the device (the host assembles their rows as zeros).
  - Shard tokens across the 8 cores BY TABLE ROW RANGE: core c owns rows
    [c*37504, (c+1)*37504). The host hands each core a contiguous slice of
    the table ("twin", 37632 rows) as its per-core input, so all gather
    windows have static bases. ~3072 typed tokens land on each core.
  - Device (per core): the HW gather (InstDMAGatherAnt) takes int16 indices,
    so the 37632-row band is covered by two windows (0:32768 and
    32768:37632). Four dma_gather instructions (3x1024 + 1x768 slots) pull
    the rows into one SBUF buffer; one contiguous DMA stores it to DRAM.
    Unused slots are padded with index 0 (a benign in-band row) and their
    output is simply discarded by the host.
  - Host: buckets/sorts tokens by row (ascending - also gives the DMA
    ascending HBM addresses), pads buckets to the fixed caps, unpermutes the
    result while assembling the full [B, S, E] output.

dma_gather layout facts (verified on HW):
  - indices live at idxs[i % 16, i // 16], int16, replicated across all 128
    partitions; valid index i lands at dst[i % 128, i // 128, :].
  - one instruction must stay near ~1024 indices (the SWDGE descriptor ring
    is small; 1792-index gathers crash the exec unit).
  - the store view maps SBUF (p, block b) -> DRAM row p*NB + b, so the DRAM
    row for slot j of a window at block base B0 is (j % 128)*NB + B0 + j//128.
"""

import numpy as np

V = 100000
E = 128
B = 16
S = 2048
NCORES = 8
P = 128

N_TOK = B * S  # 32768
NROWS = 3 * V  # 300000

RSPAN = 37504  # table rows owned per core (8 * 37504 >= 300000)
TWLEN = 37632  # per-core table slice length (RSPAN + 128 alignment margin)
W0 = 32768  # window 0 covers twin[0:32768]
W1LEN = TWLEN - W0  # 4864 rows in window 1

# Gather instructions per core: slots per instruction (multiples of 128).
# Window 0 expected ~2685 typed tokens (cap 3072), window 1 expected ~390
# (cap 768); caps sit >7 sigma above the means for the uniform input law.
W0CAP = 3072
W1CAP = 512
# (window, slot cap, swdge queue): queues 1-3 only — queue 0 runs descriptor
# generation inline on the engine (measured 8.7us blocking); queues 1+ hand
# it to the async SWDGE context so issue returns in ~60ns.
GATHERS = [(0, 1024, 1), (0, 1024, 2), (0, 1024, 3), (1, 512, 1)]
SUMCAP = W0CAP + W1CAP  # 3584
NB = SUMCAP // P  # 28 blocks
W0BLOCKS = W0CAP // P  # 24

_CACHED_NC = None


def _build_bass():
    global _CACHED_NC
    if _CACHED_NC is not None:
        return _CACHED_NC

    import concourse.bacc as bacc
    import concourse.mybir as mybir
    from concourse.library_config import mlp

    # Raw Bacc Block (no Tile): explicit semaphores avoid Tile's multi-engine
    # teardown barrier cascade (~9us) and most of its sem-clear preamble.
    # Bacc.finalize() still runs generate_event_semaphores for the 1-wait-
    # per-instruction HW limit. 4 SWDGE queues for async descriptor gen.
    nc = bacc.Bacc(num_swdge_queues=4)
    twin = nc.dram_tensor("twin", [TWLEN, E], mybir.dt.float32, kind="ExternalInput")
    idx = nc.dram_tensor("idx", [P, SUMCAP // 16], mybir.dt.int16, kind="ExternalInput")
    out = nc.dram_tensor("out", [SUMCAP, E], mybir.dt.float32, kind="ExternalOutput")

    # SBUF (p, block b) <-> DRAM row p*NB + b
    out_v = out.rearrange("(p b) e -> p (b e)", p=P)

    with (
        nc.Block() as block,
        nc.sbuf_tensor("dst", [P, NB * E], mybir.dt.float32) as dst,
        nc.sbuf_tensor("idxs", [P, SUMCAP // 16], mybir.dt.int16) as idxs,
        nc.semaphore("io") as io,
        nc.semaphore("gsem") as gsem,
    ):

        @block.gpsimd
        def _(gpsimd):
            gpsimd.load_library(mlp)
            gpsimd.dma_start(out=idxs[:], in_=idx[:]).then_inc(io, 16)
            gpsimd.wait_ge(io, 16)
            off16 = 0
            b0 = 0
            for w, cap, qn in GATHERS:
                bw = cap // P
                in_ap = twin[0:W0, :] if w == 0 else twin[W0:TWLEN, :]
                d3 = dst[:, b0 * E : (b0 + bw) * E].rearrange("p (b e) -> p b e", e=E)
                gpsimd.dma_gather(
                    d3,
                    in_ap,
                    idxs[:, off16 : off16 + cap // 16],
                    cap,
                    cap,
                    E,
                    queue_num=qn,
                ).then_inc(gsem, 16)
                off16 += cap // 16
                b0 += bw

        @block.sync
        def _(sync):
            sync.wait_ge(gsem, 16 * len(GATHERS))
            sync.dma_start(out=out_v[:], in_=dst[:]).then_inc(io, 16)
            sync.wait_ge(io, 32)

    nc.finalize()
    _CACHED_NC = nc
    return nc


def _shard_inputs(proc_emb, med_emb, chart_emb, concept, token_type):
    """Returns (in_maps, plans, tables) with per-core slot bookkeeping."""
    tables = np.ascontiguousarray(
        np.concatenate(
            [
                np.asarray(proc_emb, dtype=np.float32),
                np.asarray(med_emb, dtype=np.float32),
                np.asarray(chart_emb, dtype=np.float32),
            ],
            axis=0,
        )
    )
    tt = np.asarray(token_type).reshape(-1).astype(np.int64)
    cc = np.asarray(concept).reshape(-1).astype(np.int64)
    typed = (tt >= 1) & (tt <= 3)
    toks_all = np.where(typed)[0]  # global token ids with a real lookup
    eff = cc[toks_all] + (tt[toks_all] - 1) * V  # their table rows

    core_of = eff // RSPAN
    local = eff - core_of * RSPAN

    in_maps = []
    plans = []  # per core: (tokens, dram_rows, overflow_tokens, overflow_rows)
    for c in range(NCORES):
        base = c * RSPAN
        sl = tables[base : min(base + TWLEN, NROWS)]
        if sl.shape[0] < TWLEN:
            sl = np.concatenate([sl, np.zeros((TWLEN - sl.shape[0], E), np.float32)])
        twin = np.ascontiguousarray(sl)

        sel = np.where(core_of == c)[0]
        order = sel[np.argsort(local[sel], kind="stable")]
        lrows = local[order]  # ascending
        n0 = int(np.searchsorted(lrows, W0))  # tokens in window 0
        win_lists = [
            (lrows[:n0], toks_all[order[:n0]], W0CAP, 0, 0),
            (lrows[n0:] - W0, toks_all[order[n0:]], W1CAP, W0CAP, W0BLOCKS),
        ]

        idx16 = np.zeros((16, SUMCAP // 16), dtype=np.int16)
        tok_list, row_list, ovf_toks, ovf_rows = [], [], [], []
        for wrows, wtoks, cap, slot0, b0 in win_lists:
            cnt = len(wrows)
            if cnt > cap:
                # Statistical-tail safety valve: gather the overflow on host.
                ovf_toks.extend(wtoks[cap:].tolist())
                ovf_rows.extend((wrows[cap:] + (0 if slot0 == 0 else W0)).tolist())
                wrows, wtoks, cnt = wrows[:cap], wtoks[:cap], cap
            vals = np.zeros(cap, dtype=np.int16)
            vals[:cnt] = wrows.astype(np.int16)  # pad keeps 0 (benign row)
            idx16[:, slot0 // 16 : (slot0 + cap) // 16] = vals.reshape(cap // 16, 16).T
            j = np.arange(cnt)
            row_list.append((j % P) * NB + b0 + j // P)
            tok_list.append(wtoks)

        in_maps.append(
            {"twin": twin, "idx": np.ascontiguousarray(np.tile(idx16, (8, 1)))}
        )
        plans.append(
            (
                np.concatenate(tok_list),
                np.concatenate(row_list),
                np.array(ovf_toks, dtype=np.int64),
                np.array(ovf_rows, dtype=np.int64) + base,
            )
        )

    return in_maps, plans, tables


def _run(in_maps, trace=False):
    from concourse.bass_utils import run_bass_kernel_spmd

    nc = _build_bass()
    return run_bass_kernel_spmd(nc, in_maps, list(range(NCORES)), trace=trace)


def _assemble(results, plans, tables):
    out = np.zeros((N_TOK, E), dtype=np.float32)
    for c in range(NCORES):
        toks, drows, ovf_toks, ovf_rows = plans[c]
        if len(toks):
            out[toks] = results[c]["out"][drows]
        if len(ovf_toks):
            out[ovf_toks] = tables[ovf_rows]
    return out.reshape(B, S, E)


def kernel(proc_emb, med_emb, chart_emb, concept, token_type):
    in_maps, plans, tables = _shard_inputs(
        proc_emb, med_emb, chart_emb, concept, token_type
    )
    res = _run(in_maps, trace=False)
    return _assemble(res.results, plans, tables)



# revision 2
# speedup vs baseline: 1.2244x; 1.2244x over previous
"""Trainium2 Bass kernel for nn_ConceptEmbedding (type-conditioned embedding lookup).

Reference computation (per token position (b, s)):
    t = token_type[b, s]
    out[b, s, :] = proc_emb[concept]  if t == 1
                   med_emb[concept]   if t == 2
                   chart_emb[concept] if t == 3
                   0                  otherwise

Strategy (v4):
  - Fold the three tables into one [3V, E] table with flat row index
    (t-1)*V + concept. Tokens with t outside {1,2,3} produce zeros and are
    never sent to the device (the host assembles their rows as zeros).
  - Shard tokens across the 8 cores BY TABLE ROW RANGE: core c owns rows
    [c*37504, (c+1)*37504). The host hands each core a contiguous slice of
    the table ("twin") as its per-core input, so all gather windows have
    static bases. ~3072 typed tokens land on each core.
  - Device (per core): 4 dma_gather instructions pull rows into one SBUF
    buffer; 4 chunked dma_start stores push each gather's slab to DRAM as
    soon as that gather completes (overlapping the remaining gathers).
  - v4 critical-path changes (trace-driven, baseline 48.9us):
      * idx load issued by the Sync engine (HWDGE) as its first instruction,
        so it overlaps the ~8.7us mlp library load instead of serializing
        after it.
      * OVERLAPPING gather windows: window A = twin[0:32768] (int16 idx
        covers rows < 32768), window B = twin[5504:38272] (idx = row-5504
        covers rows 5504..38271 >= everything above A). The host splits the
        ASCENDING-sorted row list by COUNT: the largest 512 rows go to the
        window-B gather, the rest (<=2688, all < 32768) are split evenly
        over three window-A gathers. Count-based splitting removes the
        statistical variance that forced fat caps: 3200 slots vs 3584.
      * desc-gen parallelism 4-wide: SWDGE gen costs ~8.5ns/desc per
        context; queues 1-3 gen asynchronously on parallel contexts and the
        window-B gather runs INLINE on queue 0 (blocks the gpsimd engine,
        which has nothing else to do) as a 4th parallel generator.
      * per-gather stores: sync waits each gather's own semaphore and
        stores that slab immediately instead of waiting for all gathers.

dma_gather layout facts (verified on HW):
  - indices live at idxs[i % 16, i // 16], int16, replicated across all 128
    partitions; valid index i lands at dst[i % 128, i // 128, :].
  - one instruction must stay near ~1024 indices (the SWDGE descriptor ring
    is small; 1792-index gathers crash the exec unit).
  - the store view maps SBUF (p, block b) -> DRAM row p*NB + b, so the DRAM
    row for slot j of a gather at block base B0 is (j % 128)*NB + B0 + j//128.
"""

import numpy as np

V = 100000
E = 128
B = 16
S = 2048
NCORES = 8
P = 128

N_TOK = B * S  # 32768
NROWS = 3 * V  # 300000

RSPAN = 37504  # table rows owned per core (8 * 37504 >= 300000)
WB_BASE = 5504  # window B covers twin[5504:38272] (idx = row - 5504)
TWLEN = WB_BASE + 32768  # 38272 rows: window B must stay in-bounds

# Gather slots: three window-A gathers (queues 1-3, async contexts) and one
# window-B gather (queue 0, inline gen - 4th parallel generator).
CAP_A = 896
CAP_B = 512
GATHERS = [  # (cap, slot0, window_base, queue)
    (CAP_A, 0, 0, 1),
    (CAP_A, CAP_A, 0, 2),
    (CAP_A, 2 * CAP_A, 0, 3),
    (CAP_B, 3 * CAP_A, WB_BASE, 0),
]
SUMCAP = 3 * CAP_A + CAP_B  # 3200
NB = SUMCAP // P  # 25 blocks

_CACHED_NC = None


def _build_bass():
    global _CACHED_NC
    if _CACHED_NC is not None:
        return _CACHED_NC

    import concourse.bacc as bacc
    import concourse.mybir as mybir
    from concourse.library_config import mlp

    # Raw Bacc Block (no Tile): explicit semaphores avoid Tile's multi-engine
    # teardown barrier cascade (~9us) and most of its sem-clear preamble.
    nc = bacc.Bacc(num_swdge_queues=4)
    twin = nc.dram_tensor("twin", [TWLEN, E], mybir.dt.float32, kind="ExternalInput")
    idx = nc.dram_tensor("idx", [P, SUMCAP // 16], mybir.dt.int16, kind="ExternalInput")
    out = nc.dram_tensor("out", [SUMCAP, E], mybir.dt.float32, kind="ExternalOutput")

    # SBUF (p, block b) <-> DRAM row p*NB + b
    out_v = out.rearrange("(p b) e -> p (b e)", p=P)

    with (
        nc.Block() as block,
        nc.sbuf_tensor("dst", [P, NB * E], mybir.dt.float32) as dst,
        nc.sbuf_tensor("idxs", [P, SUMCAP // 16], mybir.dt.int16) as idxs,
        nc.semaphore("io") as io,
        nc.semaphore("g0") as g0,
        nc.semaphore("g1") as g1,
        nc.semaphore("g2") as g2,
        nc.semaphore("g3") as g3,
    ):
        gsems = [g0, g1, g2, g3]

        @block.sync
        def _(sync):
            # idx load on HWDGE: overlaps the gpsimd library load.
            sync.dma_start(out=idxs[:], in_=idx[:]).then_inc(io, 16)
            # Store each gather's slab as soon as that gather lands.
            for k, (cap, slot0, _wb, _q) in enumerate(GATHERS):
                b0, bw = slot0 // P, cap // P
                sync.wait_ge(gsems[k], 16)
                sync.dma_start(
                    out=out_v[:, b0 * E : (b0 + bw) * E],
                    in_=dst[:, b0 * E : (b0 + bw) * E],
                ).then_inc(io, 16)
            sync.wait_ge(io, 16 * (1 + len(GATHERS)))

        @block.gpsimd
        def _(gpsimd):
            gpsimd.load_library(mlp)
            gpsimd.wait_ge(io, 16)  # idx in SBUF
            for k, (cap, slot0, wb, qn) in enumerate(GATHERS):
                b0, bw = slot0 // P, cap // P
                in_ap = twin[wb : wb + 32768, :]
                d3 = dst[:, b0 * E : (b0 + bw) * E].rearrange("p (b e) -> p b e", e=E)
                gpsimd.dma_gather(
                    d3,
                    in_ap,
                    idxs[:, slot0 // 16 : (slot0 + cap) // 16],
                    cap,
                    cap,
                    E,
                    queue_num=qn,
                ).then_inc(gsems[k], 16)

    nc.finalize()
    _CACHED_NC = nc
    return nc


def _shard_inputs(proc_emb, med_emb, chart_emb, concept, token_type):
    """Returns (in_maps, plans, tables) with per-core slot bookkeeping."""
    tables = np.ascontiguousarray(
        np.concatenate(
            [
                np.asarray(proc_emb, dtype=np.float32),
                np.asarray(med_emb, dtype=np.float32),
                np.asarray(chart_emb, dtype=np.float32),
            ],
            axis=0,
        )
    )
    tt = np.asarray(token_type).reshape(-1).astype(np.int64)
    cc = np.asarray(concept).reshape(-1).astype(np.int64)
    typed = (tt >= 1) & (tt <= 3)
    toks_all = np.where(typed)[0]  # global token ids with a real lookup
    eff = cc[toks_all] + (tt[toks_all] - 1) * V  # their table rows

    core_of = eff // RSPAN
    local = eff - core_of * RSPAN

    in_maps = []
    plans = []  # per core: (tokens, dram_rows, overflow_tokens, overflow_rows)
    for c in range(NCORES):
        base = c * RSPAN
        sl = tables[base : min(base + TWLEN, NROWS)]
        if sl.shape[0] < TWLEN:
            sl = np.concatenate([sl, np.zeros((TWLEN - sl.shape[0], E), np.float32)])
        twin = np.ascontiguousarray(sl)

        sel = np.where(core_of == c)[0]
        order = sel[np.argsort(local[sel], kind="stable")]
        lrows = local[order]  # ascending
        gtoks = toks_all[order]
        n = len(lrows)

        # Window-B gather takes the largest CAP_B rows (they must be >= 5504
        # to fit window B; everything >= 32768 MUST land there). Rest goes
        # ascending into the three window-A gathers (< 32768 required).
        # Count-overflow tails are gathered on the host.
        ovf_toks, ovf_rows = [], []
        n_hi = int(n - np.searchsorted(lrows, 32768))  # rows >= 32768
        if n_hi > CAP_B:  # window-B capacity overflow -> host
            spill = n_hi - CAP_B
            ovf_toks.extend(gtoks[n - spill :].tolist())
            ovf_rows.extend(lrows[n - spill :].tolist())
            lrows, gtoks, n = lrows[:-spill], gtoks[:-spill], n - spill
            n_hi = CAP_B
        # fill window B up to CAP_B with the largest remaining rows >= WB_BASE
        n_b = min(CAP_B, n - int(np.searchsorted(lrows, WB_BASE)))
        n_a = n - n_b
        if n_a > 3 * CAP_A:  # window-A capacity overflow -> host
            spill = n_a - 3 * CAP_A
            # drop the largest window-A rows to the host
            ovf_toks.extend(gtoks[n_a - spill : n_a].tolist())
            ovf_rows.extend(lrows[n_a - spill : n_a].tolist())
            lrows = np.concatenate([lrows[: n_a - spill], lrows[n_a:]])
            gtoks = np.concatenate([gtoks[: n_a - spill], gtoks[n_a:]])
            n, n_a = n - spill, 3 * CAP_A

        # per-gather (rows, tokens): window A split evenly, window B last.
        cuts = [
            (n_a + 2) // 3,
            (n_a + 1) // 3 + (n_a + 2) // 3,
            n_a,
            n,
        ]
        idx16 = np.zeros((16, SUMCAP // 16), dtype=np.int16)
        tok_list, row_list = [], []
        lo = 0
        for (cap, slot0, wb, _q), hi in zip(GATHERS, cuts):
            wrows, wtoks = lrows[lo:hi], gtoks[lo:hi]
            lo = hi
            cnt = len(wrows)
            vals = np.zeros(cap, dtype=np.int16)
            vals[:cnt] = (wrows - wb).astype(np.int16)  # pad 0 = benign row
            idx16[:, slot0 // 16 : (slot0 + cap) // 16] = (
                vals.reshape(cap // 16, 16).T
            )
            j = np.arange(cnt)
            row_list.append((j % P) * NB + slot0 // P + j // P)
            tok_list.append(wtoks)

        in_maps.append(
            {"twin": twin, "idx": np.ascontiguousarray(np.tile(idx16, (8, 1)))}
        )
        plans.append(
            (
                np.concatenate(tok_list),
                np.concatenate(row_list),
                np.array(ovf_toks, dtype=np.int64),
                np.array(ovf_rows, dtype=np.int64) + base,
            )
        )

    return in_maps, plans, tables


def _run(in_maps, trace=False):
    from concourse.bass_utils import run_bass_kernel_spmd

    nc = _build_bass()
    return run_bass_kernel_spmd(nc, in_maps, list(range(NCORES)), trace=trace)


def _assemble(results, plans, tables):
    out = np.zeros((N_TOK, E), dtype=np.float32)
    for c in range(NCORES):
        toks, drows, ovf_toks, ovf_rows = plans[c]
        if len(toks):
            out[toks] = results[c]["out"][drows]
        if len(ovf_toks):
            out[ovf_toks] = tables[ovf_rows]
    return out.reshape(B, S, E)


def kernel(proc_emb, med_emb, chart_emb, concept, token_type):
    in_maps, plans, tables = _shard_inputs(
        proc_emb, med_emb, chart_emb, concept, token_type
    )
    res = _run(in_maps, trace=False)
    return _assemble(res.results, plans, tables)


# revision 5
# speedup vs baseline: 1.2993x; 1.0611x over previous
"""Trainium2 Bass kernel for nn_ConceptEmbedding (type-conditioned embedding lookup).

Reference computation (per token position (b, s)):
    t = token_type[b, s]
    out[b, s, :] = proc_emb[concept]  if t == 1
                   med_emb[concept]   if t == 2
                   chart_emb[concept] if t == 3
                   0                  otherwise

Strategy (v4):
  - Fold the three tables into one [3V, E] table with flat row index
    (t-1)*V + concept. Tokens with t outside {1,2,3} produce zeros and are
    never sent to the device (the host assembles their rows as zeros).
  - Shard tokens across the 8 cores BY TABLE ROW RANGE: core c owns rows
    [c*37504, (c+1)*37504). The host hands each core a contiguous slice of
    the table ("twin") as its per-core input, so all gather windows have
    static bases. ~3072 typed tokens land on each core.
  - Device (per core): 4 dma_gather instructions pull rows into one SBUF
    buffer; 4 chunked dma_start stores push each gather's slab to DRAM as
    soon as that gather completes (overlapping the remaining gathers).
  - v4 critical-path changes (trace-driven, baseline 48.9us):
      * idx load issued by the Sync engine (HWDGE) as its first instruction,
        so it overlaps the ~8.7us mlp library load instead of serializing
        after it.
      * OVERLAPPING gather windows: window A = twin[0:32768] (int16 idx
        covers rows < 32768), window B = twin[5504:38272] (idx = row-5504
        covers rows 5504..38271 >= everything above A). The host splits the
        ASCENDING-sorted row list by COUNT: the largest 512 rows go to the
        window-B gather, the rest (<=2688, all < 32768) are split evenly
        over three window-A gathers. Count-based splitting removes the
        statistical variance that forced fat caps: 3200 slots vs 3584.
      * desc-gen parallelism 4-wide: SWDGE gen costs ~8.5ns/desc per
        context; queues 1-3 gen asynchronously on parallel contexts and the
        window-B gather runs INLINE on queue 0 (blocks the gpsimd engine,
        which has nothing else to do) as a 4th parallel generator.
      * per-gather stores: sync waits each gather's own semaphore and
        stores that slab immediately instead of waiting for all gathers.

dma_gather layout facts (verified on HW):
  - indices live at idxs[i % 16, i // 16], int16, replicated across all 128
    partitions; valid index i lands at dst[i % 128, i // 128, :].
  - one instruction must stay near ~1024 indices (the SWDGE descriptor ring
    is small; 1792-index gathers crash the exec unit).
  - the store view maps SBUF (p, block b) -> DRAM row p*NB + b, so the DRAM
    row for slot j of a gather at block base B0 is (j % 128)*NB + B0 + j//128.
"""

import numpy as np

V = 100000
E = 128
B = 16
S = 2048
NCORES = 8
P = 128

N_TOK = B * S  # 32768
NROWS = 3 * V  # 300000

RSPAN = 37504  # table rows owned per core (8 * 37504 >= 300000)
WB_BASE = 5504  # window B covers twin[5504:38272] (idx = row - 5504)
TWLEN = WB_BASE + 32768  # 38272 rows: window B must stay in-bounds

# Gather slots: three window-A gathers (queues 1-3, async contexts) and one
# window-B gather (queue 0, inline gen - 4th parallel generator). Sizes are
# STAGGERED so desc-gen completions (and hence transfers and stores) spread
# out instead of bunching: gen costs ~7.6ns/desc per context.
CAP_B = 512
GATHERS = [  # (cap, slot0, window_base, queue) - slot order = SBUF block order
    (CAP_B, 0, WB_BASE, 0),  # blocks 0-3, inline gen, finishes first
    (768, 512, 0, 1),  # blocks 4-9
    (896, 1280, 0, 2),  # blocks 10-16
    (1024, 2176, 0, 3),  # blocks 17-24
]
CAPS_A = [c for c, _s, w, _q in GATHERS if w == 0]
SUMCAP = sum(c for c, _s, _w, _q in GATHERS)  # 3200
NB = SUMCAP // P  # 25 blocks
# Two stores, each one contiguous block range (big per-partition descriptors:
# DMA engine 15 pays a fixed per-packet penalty, so fewer/larger packets).
# store-A = gathers 0+1 (blocks 0-9), store-B = gathers 2+3 (blocks 10-24).
STORES = [((0, 1), 0, 10), ((2, 3), 10, 25)]  # (gather ids, block lo, block hi)

_CACHED_NC = None


def _build_bass():
    global _CACHED_NC
    if _CACHED_NC is not None:
        return _CACHED_NC

    import concourse.bacc as bacc
    import concourse.mybir as mybir
    from concourse.library_config import mlp

    # Raw Bacc Block (no Tile): explicit semaphores avoid Tile's multi-engine
    # teardown barrier cascade (~9us) and most of its sem-clear preamble.
    nc = bacc.Bacc(num_swdge_queues=4)
    twin = nc.dram_tensor("twin", [TWLEN, E], mybir.dt.float32, kind="ExternalInput")
    idx = nc.dram_tensor("idx", [P, SUMCAP // 16], mybir.dt.int16, kind="ExternalInput")
    out = nc.dram_tensor("out", [SUMCAP, E], mybir.dt.float32, kind="ExternalOutput")

    # SBUF (p, block b) <-> DRAM row p*NB + b
    out_v = out.rearrange("(p b) e -> p (b e)", p=P)

    with (
        nc.Block() as block,
        nc.sbuf_tensor("dst", [P, NB * E], mybir.dt.float32) as dst,
        nc.sbuf_tensor("idxs", [P, SUMCAP // 16], mybir.dt.int16) as idxs,
        nc.semaphore("io") as io,
        nc.semaphore("g0") as g0,
        nc.semaphore("g1") as g1,
        nc.semaphore("g2") as g2,
        nc.semaphore("g3") as g3,
    ):
        gsems = [g0, g1, g2, g3]

        @block.sync
        def _(sync):
            # idx load on HWDGE: overlaps the gpsimd library load.
            sync.dma_start(out=idxs[:], in_=idx[:]).then_inc(io, 16)
            # Each store fires once its pair of gathers lands.
            for gids, blo, bhi in STORES:
                for g in gids:
                    sync.wait_ge(gsems[g], 16)
                sync.dma_start(
                    out=out_v[:, blo * E : bhi * E],
                    in_=dst[:, blo * E : bhi * E],
                ).then_inc(io, 16)
            sync.wait_ge(io, 16 * (1 + len(STORES)))

        @block.gpsimd
        def _(gpsimd):
            gpsimd.load_library(mlp)
            gpsimd.wait_ge(io, 16)  # idx in SBUF
            # async queues first (issue returns fast), inline queue-0 last
            # (it blocks the engine for its whole gen).
            order = [k for k, g in enumerate(GATHERS) if g[3] != 0] + [
                k for k, g in enumerate(GATHERS) if g[3] == 0
            ]
            for k in order:
                cap, slot0, wb, qn = GATHERS[k]
                b0, bw = slot0 // P, cap // P
                in_ap = twin[wb : wb + 32768, :]
                d3 = dst[:, b0 * E : (b0 + bw) * E].rearrange("p (b e) -> p b e", e=E)
                gpsimd.dma_gather(
                    d3,
                    in_ap,
                    idxs[:, slot0 // 16 : (slot0 + cap) // 16],
                    cap,
                    cap,
                    E,
                    queue_num=qn,
                ).then_inc(gsems[k], 16)

    nc.finalize()
    _CACHED_NC = nc
    return nc


def _shard_inputs(proc_emb, med_emb, chart_emb, concept, token_type):
    """Returns (in_maps, plans, tables) with per-core slot bookkeeping."""
    tables = np.ascontiguousarray(
        np.concatenate(
            [
                np.asarray(proc_emb, dtype=np.float32),
                np.asarray(med_emb, dtype=np.float32),
                np.asarray(chart_emb, dtype=np.float32),
            ],
            axis=0,
        )
    )
    tt = np.asarray(token_type).reshape(-1).astype(np.int64)
    cc = np.asarray(concept).reshape(-1).astype(np.int64)
    typed = (tt >= 1) & (tt <= 3)
    toks_all = np.where(typed)[0]  # global token ids with a real lookup
    eff = cc[toks_all] + (tt[toks_all] - 1) * V  # their table rows

    core_of = eff // RSPAN
    local = eff - core_of * RSPAN

    in_maps = []
    plans = []  # per core: (tokens, dram_rows, overflow_tokens, overflow_rows)
    for c in range(NCORES):
        base = c * RSPAN
        sl = tables[base : min(base + TWLEN, NROWS)]
        if sl.shape[0] < TWLEN:
            sl = np.concatenate([sl, np.zeros((TWLEN - sl.shape[0], E), np.float32)])
        twin = np.ascontiguousarray(sl)

        sel = np.where(core_of == c)[0]
        order = sel[np.argsort(local[sel], kind="stable")]
        lrows = local[order]  # ascending
        gtoks = toks_all[order]
        n = len(lrows)

        # Window-B gather takes the largest CAP_B rows (they must be >= 5504
        # to fit window B; everything >= 32768 MUST land there). Rest goes
        # ascending into the three window-A gathers (< 32768 required),
        # filled sequentially. Count-overflow tails are gathered on the host.
        ovf_toks, ovf_rows = [], []
        n_hi = int(n - np.searchsorted(lrows, 32768))  # rows >= 32768
        if n_hi > CAP_B:  # window-B capacity overflow -> host
            spill = n_hi - CAP_B
            ovf_toks.extend(gtoks[n - spill :].tolist())
            ovf_rows.extend(lrows[n - spill :].tolist())
            lrows, gtoks, n = lrows[:-spill], gtoks[:-spill], n - spill
            n_hi = CAP_B
        # fill window B up to CAP_B with the largest remaining rows >= WB_BASE
        n_b = min(CAP_B, n - int(np.searchsorted(lrows, WB_BASE)))
        n_a = n - n_b
        if n_a > sum(CAPS_A):  # window-A capacity overflow -> host
            spill = n_a - sum(CAPS_A)
            # drop the largest window-A rows to the host
            ovf_toks.extend(gtoks[n_a - spill : n_a].tolist())
            ovf_rows.extend(lrows[n_a - spill : n_a].tolist())
            lrows = np.concatenate([lrows[: n_a - spill], lrows[n_a:]])
            gtoks = np.concatenate([gtoks[: n_a - spill], gtoks[n_a:]])
            n, n_a = n - spill, sum(CAPS_A)

        # per-gather (lo, hi) in the sorted row list: gather 0 (window B)
        # takes the tail [n_a:n]; gathers 1..3 fill sequentially.
        spans = [(n_a, n)]
        lo = 0
        for cap in CAPS_A:
            hi = min(lo + cap, n_a)
            spans.append((lo, hi))
            lo = hi
        idx16 = np.zeros((16, SUMCAP // 16), dtype=np.int16)
        tok_list, row_list = [], []
        for (cap, slot0, wb, _q), (lo, hi) in zip(GATHERS, spans):
            wrows, wtoks = lrows[lo:hi], gtoks[lo:hi]
            cnt = len(wrows)
            vals = np.zeros(cap, dtype=np.int16)
            vals[:cnt] = (wrows - wb).astype(np.int16)  # pad 0 = benign row
            idx16[:, slot0 // 16 : (slot0 + cap) // 16] = (
                vals.reshape(cap // 16, 16).T
            )
            j = np.arange(cnt)
            row_list.append((j % P) * NB + slot0 // P + j // P)
            tok_list.append(wtoks)

        in_maps.append(
            {"twin": twin, "idx": np.ascontiguousarray(np.tile(idx16, (8, 1)))}
        )
        plans.append(
            (
                np.concatenate(tok_list),
                np.concatenate(row_list),
                np.array(ovf_toks, dtype=np.int64),
                np.array(ovf_rows, dtype=np.int64) + base,
            )
        )

    return in_maps, plans, tables


def _run(in_maps, trace=False):
    from concourse.bass_utils import run_bass_kernel_spmd

    nc = _build_bass()
    return run_bass_kernel_spmd(nc, in_maps, list(range(NCORES)), trace=trace)


def _assemble(results, plans, tables):
    out = np.zeros((N_TOK, E), dtype=np.float32)
    for c in range(NCORES):
        toks, drows, ovf_toks, ovf_rows = plans[c]
        if len(toks):
            out[toks] = results[c]["out"][drows]
        if len(ovf_toks):
            out[ovf_toks] = tables[ovf_rows]
    return out.reshape(B, S, E)


def kernel(proc_emb, med_emb, chart_emb, concept, token_type):
    in_maps, plans, tables = _shard_inputs(
        proc_emb, med_emb, chart_emb, concept, token_type
    )
    res = _run(in_maps, trace=False)
    return _assemble(res.results, plans, tables)


# revision 8
# speedup vs baseline: 1.4440x; 1.1114x over previous
"""Trainium2 Bass kernel for nn_ConceptEmbedding (type-conditioned embedding lookup).

Reference computation (per token position (b, s)):
    t = token_type[b, s]
    out[b, s, :] = proc_emb[concept]  if t == 1
                   med_emb[concept]   if t == 2
                   chart_emb[concept] if t == 3
                   0                  otherwise

Strategy (v4):
  - Fold the three tables into one [3V, E] table with flat row index
    (t-1)*V + concept. Tokens with t outside {1,2,3} produce zeros and are
    never sent to the device (the host assembles their rows as zeros).
  - Shard tokens across the 8 cores BY TABLE ROW RANGE: core c owns rows
    [c*37504, (c+1)*37504). The host hands each core a contiguous slice of
    the table ("twin") as its per-core input, so all gather windows have
    static bases. ~3072 typed tokens land on each core.
  - Device (per core): 4 dma_gather instructions pull rows into one SBUF
    buffer; 4 chunked dma_start stores push each gather's slab to DRAM as
    soon as that gather completes (overlapping the remaining gathers).
  - v4 critical-path changes (trace-driven, baseline 48.9us):
      * idx load issued by the Sync engine (HWDGE) as its first instruction,
        so it overlaps the ~8.7us mlp library load instead of serializing
        after it.
      * OVERLAPPING gather windows: window A = twin[0:32768] (int16 idx
        covers rows < 32768), window B = twin[5504:38272] (idx = row-5504
        covers rows 5504..38271 >= everything above A). The host splits the
        ASCENDING-sorted row list by COUNT: the largest 512 rows go to the
        window-B gather, the rest (<=2688, all < 32768) are split evenly
        over three window-A gathers. Count-based splitting removes the
        statistical variance that forced fat caps: 3200 slots vs 3584.
      * desc-gen parallelism 4-wide: SWDGE gen costs ~8.5ns/desc per
        context; queues 1-3 gen asynchronously on parallel contexts and the
        window-B gather runs INLINE on queue 0 (blocks the gpsimd engine,
        which has nothing else to do) as a 4th parallel generator.
      * per-gather stores: sync waits each gather's own semaphore and
        stores that slab immediately instead of waiting for all gathers.

dma_gather layout facts (verified on HW):
  - indices live at idxs[i % 16, i // 16], int16, replicated across all 128
    partitions; valid index i lands at dst[i % 128, i // 128, :].
  - one instruction must stay near ~1024 indices (the SWDGE descriptor ring
    is small; 1792-index gathers crash the exec unit).
  - the store view maps SBUF (p, block b) -> DRAM row p*NB + b, so the DRAM
    row for slot j of a gather at block base B0 is (j % 128)*NB + B0 + j//128.
"""

import numpy as np

V = 100000
E = 128
B = 16
S = 2048
NCORES = 8
P = 128

N_TOK = B * S  # 32768
NROWS = 3 * V  # 300000

RSPAN = 37504  # table rows owned per core (8 * 37504 >= 300000)
WB_BASE = 5504  # window B covers twin[5504:38272] (idx = row - 5504)
TWLEN = WB_BASE + 32768  # 38272 rows: window B must stay in-bounds

# Gather slots: window-B gather on queue 0 (inline gen, ~12ns/desc) plus two
# ROUNDS of window-A gathers on async queues 1-3 (~7.4ns/desc per context,
# ~1us fixed per instruction). Two rounds start the DMA-engine drain ~4us
# earlier than one big gather per context: DMA engine 15 runs gather packets
# at ~half speed, so its drain backlog is the transfer critical path and
# should start as early as possible.
CAP_B = 512
GATHERS = [  # (cap, slot0, window_base, queue) - slot order = SBUF block order
    (CAP_B, 0, WB_BASE, 0),  # blocks 0-3, inline
    (512, 512, 0, 1),  # round a: blocks 4-15
    (512, 1024, 0, 2),
    (512, 1536, 0, 3),
    (384, 2048, 0, 1),  # round b: blocks 16-24
    (384, 2432, 0, 2),
    (384, 2816, 0, 3),
]
CAPS_A = [c for c, _s, w, _q in GATHERS if w == 0]
SUMCAP = sum(c for c, _s, _w, _q in GATHERS)  # 3200
NB = SUMCAP // P  # 25 blocks
# Two stores, each one contiguous block range (big per-partition descriptors:
# DMA engine 15 pays a fixed per-packet penalty, so fewer/larger packets).
STORES = [((0, 1, 2, 3), 0, 16), ((4, 5, 6), 16, 25)]  # (gather ids, blk lo, hi)

_CACHED_NC = None


def _build_bass():
    global _CACHED_NC
    if _CACHED_NC is not None:
        return _CACHED_NC

    import concourse.bacc as bacc
    import concourse.mybir as mybir
    from concourse.library_config import mlp

    from contextlib import ExitStack

    # Raw Bacc Block (no Tile): explicit semaphores avoid Tile's multi-engine
    # teardown barrier cascade (~9us) and most of its sem-clear preamble.
    nc = bacc.Bacc(num_swdge_queues=4, monotonic_sem_count=0)
    twin = nc.dram_tensor("twin", [TWLEN, E], mybir.dt.float32, kind="ExternalInput")
    idx = nc.dram_tensor("idx", [P, SUMCAP // 16], mybir.dt.int16, kind="ExternalInput")
    out = nc.dram_tensor("out", [SUMCAP, E], mybir.dt.float32, kind="ExternalOutput")

    # SBUF (p, block b) <-> DRAM row p*NB + b
    out_v = out.rearrange("(p b) e -> p (b e)", p=P)

    with (
        ExitStack() as stack,
        nc.Block() as block,
        nc.sbuf_tensor("dst", [P, NB * E], mybir.dt.float32) as dst,
        nc.sbuf_tensor("idxs", [P, SUMCAP // 16], mybir.dt.int16) as idxs,
        nc.semaphore("io") as io,
    ):
        gsems = [
            stack.enter_context(nc.semaphore(f"g{k}")) for k in range(len(GATHERS))
        ]

        @block.sync
        def _(sync):
            # idx load on HWDGE: overlaps the gpsimd library load.
            sync.dma_start(out=idxs[:], in_=idx[:]).then_inc(io, 16)
            # Each store fires once its pair of gathers lands.
            for gids, blo, bhi in STORES:
                for g in gids:
                    sync.wait_ge(gsems[g], 16)
                sync.dma_start(
                    out=out_v[:, blo * E : bhi * E],
                    in_=dst[:, blo * E : bhi * E],
                ).then_inc(io, 16)
            sync.wait_ge(io, 16 * (1 + len(STORES)))

        @block.gpsimd
        def _(gpsimd):
            gpsimd.load_library(mlp)
            gpsimd.wait_ge(io, 16)  # idx in SBUF
            # async queues first (issue returns fast), inline queue-0 last
            # (it blocks the engine for its whole gen).
            order = [k for k, g in enumerate(GATHERS) if g[3] != 0] + [
                k for k, g in enumerate(GATHERS) if g[3] == 0
            ]
            for k in order:
                cap, slot0, wb, qn = GATHERS[k]
                b0, bw = slot0 // P, cap // P
                in_ap = twin[wb : wb + 32768, :]
                d3 = dst[:, b0 * E : (b0 + bw) * E].rearrange("p (b e) -> p b e", e=E)
                gpsimd.dma_gather(
                    d3,
                    in_ap,
                    idxs[:, slot0 // 16 : (slot0 + cap) // 16],
                    cap,
                    cap,
                    E,
                    queue_num=qn,
                ).then_inc(gsems[k], 16)

    nc.finalize()
    if CULL_UNUSED_ENGINES:
        _cull_unused_engines(nc, mybir)
    _CACHED_NC = nc
    return nc


# Drop the Tensor/Vector/Scalar engines from the compiled module: the kernel
# only uses Pool (gpsimd) and SP (sync), but Bacc emits entry/exit barrier
# instructions for all five engines, and the NEFF then carries five engine
# queues whose NRT launch/teardown handshakes sit on the critical path.
# Removing the three idle engines' barrier legs (and shrinking the Pool-side
# barrier counts from 4 peers to 1) trims the fixed preamble/teardown cost.
CULL_UNUSED_ENGINES = True


def _cull_unused_engines(nc, mybir):
    cull = {
        mybir.EngineType.PE,
        mybir.EngineType.DVE,
        mybir.EngineType.Activation,
    }
    n_peers = 1  # SP is the only remaining non-Pool engine
    for b in nc.main_func.blocks:
        kept = [
            inst for inst in b.instructions if getattr(inst, "engine", None) not in cull
        ]
        if len(kept) != len(b.instructions):
            b.instructions[:] = kept
        for inst in kept:
            si = inst.sync_info
            if si is None:
                continue
            for w in si.on_wait:
                if "barrier" in (w.ant_name or "") and w.wait_value == 4:
                    w.wait_value = n_peers
            for u in si.on_update:
                if "barrier" in (u.ant_name or "") and u.update_value == 4:
                    u.update_value = n_peers


def _shard_inputs(proc_emb, med_emb, chart_emb, concept, token_type):
    """Returns (in_maps, plans, tables) with per-core slot bookkeeping."""
    tables = np.ascontiguousarray(
        np.concatenate(
            [
                np.asarray(proc_emb, dtype=np.float32),
                np.asarray(med_emb, dtype=np.float32),
                np.asarray(chart_emb, dtype=np.float32),
            ],
            axis=0,
        )
    )
    tt = np.asarray(token_type).reshape(-1).astype(np.int64)
    cc = np.asarray(concept).reshape(-1).astype(np.int64)
    typed = (tt >= 1) & (tt <= 3)
    toks_all = np.where(typed)[0]  # global token ids with a real lookup
    eff = cc[toks_all] + (tt[toks_all] - 1) * V  # their table rows

    core_of = eff // RSPAN
    local = eff - core_of * RSPAN

    in_maps = []
    plans = []  # per core: (tokens, dram_rows, overflow_tokens, overflow_rows)
    for c in range(NCORES):
        base = c * RSPAN
        sl = tables[base : min(base + TWLEN, NROWS)]
        if sl.shape[0] < TWLEN:
            sl = np.concatenate([sl, np.zeros((TWLEN - sl.shape[0], E), np.float32)])
        twin = np.ascontiguousarray(sl)

        sel = np.where(core_of == c)[0]
        order = sel[np.argsort(local[sel], kind="stable")]
        lrows = local[order]  # ascending
        gtoks = toks_all[order]
        n = len(lrows)

        # Window-B gather takes the largest CAP_B rows (they must be >= 5504
        # to fit window B; everything >= 32768 MUST land there). Rest goes
        # ascending into the three window-A gathers (< 32768 required),
        # filled sequentially. Count-overflow tails are gathered on the host.
        ovf_toks, ovf_rows = [], []
        n_hi = int(n - np.searchsorted(lrows, 32768))  # rows >= 32768
        if n_hi > CAP_B:  # window-B capacity overflow -> host
            spill = n_hi - CAP_B
            ovf_toks.extend(gtoks[n - spill :].tolist())
            ovf_rows.extend(lrows[n - spill :].tolist())
            lrows, gtoks, n = lrows[:-spill], gtoks[:-spill], n - spill
            n_hi = CAP_B
        # fill window B up to CAP_B with the largest remaining rows >= WB_BASE
        n_b = min(CAP_B, n - int(np.searchsorted(lrows, WB_BASE)))
        n_a = n - n_b
        if n_a > sum(CAPS_A):  # window-A capacity overflow -> host
            spill = n_a - sum(CAPS_A)
            # drop the largest window-A rows to the host
            ovf_toks.extend(gtoks[n_a - spill : n_a].tolist())
            ovf_rows.extend(lrows[n_a - spill : n_a].tolist())
            lrows = np.concatenate([lrows[: n_a - spill], lrows[n_a:]])
            gtoks = np.concatenate([gtoks[: n_a - spill], gtoks[n_a:]])
            n, n_a = n - spill, sum(CAPS_A)

        # per-gather (lo, hi) in the sorted row list: gather 0 (window B)
        # takes the tail [n_a:n]; gathers 1..3 fill sequentially.
        spans = [(n_a, n)]
        lo = 0
        for cap in CAPS_A:
            hi = min(lo + cap, n_a)
            spans.append((lo, hi))
            lo = hi
        idx16 = np.zeros((16, SUMCAP // 16), dtype=np.int16)
        tok_list, row_list = [], []
        for (cap, slot0, wb, _q), (lo, hi) in zip(GATHERS, spans):
            wrows, wtoks = lrows[lo:hi], gtoks[lo:hi]
            cnt = len(wrows)
            vals = np.zeros(cap, dtype=np.int16)
            vals[:cnt] = (wrows - wb).astype(np.int16)  # pad 0 = benign row
            idx16[:, slot0 // 16 : (slot0 + cap) // 16] = (
                vals.reshape(cap // 16, 16).T
            )
            j = np.arange(cnt)
            row_list.append((j % P) * NB + slot0 // P + j // P)
            tok_list.append(wtoks)

        in_maps.append(
            {"twin": twin, "idx": np.ascontiguousarray(np.tile(idx16, (8, 1)))}
        )
        plans.append(
            (
                np.concatenate(tok_list),
                np.concatenate(row_list),
                np.array(ovf_toks, dtype=np.int64),
                np.array(ovf_rows, dtype=np.int64) + base,
            )
        )

    return in_maps, plans, tables


def _run(in_maps, trace=False):
    from concourse.bass_utils import run_bass_kernel_spmd

    nc = _build_bass()
    return run_bass_kernel_spmd(nc, in_maps, list(range(NCORES)), trace=trace)


def _assemble(results, plans, tables):
    out = np.zeros((N_TOK, E), dtype=np.float32)
    for c in range(NCORES):
        toks, drows, ovf_toks, ovf_rows = plans[c]
        if len(toks):
            out[toks] = results[c]["out"][drows]
        if len(ovf_toks):
            out[ovf_toks] = tables[ovf_rows]
    return out.reshape(B, S, E)


def kernel(proc_emb, med_emb, chart_emb, concept, token_type):
    in_maps, plans, tables = _shard_inputs(
        proc_emb, med_emb, chart_emb, concept, token_type
    )
    res = _run(in_maps, trace=False)
    return _assemble(res.results, plans, tables)
